# revision 9
# baseline (speedup 1.0000x reference)
"""DynamicRelationshipExtractor Trainium2 kernel (8 NeuronCores, batch-sharded).

Sharding: core k handles batches [8k, 8k+8) x all 2016 pairs (16128 tokens).
All cores run ONE SPMD program; per-core data enters via inputs. Pair indices
are baked into access patterns at trace time.

v2 restructure (vs the 180.8us baseline):
- Pairs re-ordered into 8x8 rectangular blocks (28 full blocks above the
  block-diagonal) plus cross-runs that batch the 8 diagonal triangles with a
  stride-64 AP dim. 32 tiles, 1-2 regular blocks each; feature elementwise
  work becomes a few large multi-dim-AP instructions per tile.
- Feature pairs sharing one ALU op are computed in single merged instructions
  over a panel-pair AP dim: (cos,mult) on GpSimd, (dab,dba) and (diff,add) on
  Vector. |diff| on Scalar. Panels [xn,xT,xr,xT,-xT,xT] are host-built so all
  pairs are forward-contiguous panel slices.
- sub/subba stage-1 is one full-M=128 matmul with lhsT [W_sub|-W_sub] reading
  the diff feature (full-M matmuls hide LDWEIGHTS; M=64 col-pairs don't).
- Chunk relu evictions merged into 2 activations of [128, 2*NT] per tile
  (zero-bias fast path; 4 split activations with bias APs otherwise).
- relu(u) eviction is one merged vector tensor_scalar over [128, 2*NT].
- Stage-3 accumulator goes PSUM -> DRAM by DMA directly (fus_b2 added on
  host), removing the final eviction pass.
"""
import sys
import numpy as np
import ml_dtypes
from contextlib import ExitStack

if '/opt/trn_rl_repo' not in sys.path:
    sys.path.insert(0, '/opt/trn_rl_repo')

import concourse.bass as bass
import concourse.tile as tile
from concourse import mybir
from concourse.bass_utils import run_bass_kernel_spmd

bf16 = mybir.dt.bfloat16
f32 = mybir.dt.float32
AF = mybir.ActivationFunctionType
ALU = mybir.AluOpType
BF = ml_dtypes.bfloat16

D = 128
NCOLS = 64
B = 64
P_TOT = 2016
EPS = 1e-6
MLP_IDX = [0, 1, 2, 3, 4, 4, 5, 5]
NCORES = 8
BC = B // NCORES            # batches per core = 8
NTOK = P_TOT * BC           # tokens per core = 16128
NPANEL = 4                  # xn, xT, xr, -xT
NCOL_DEV = NCOLS * BC       # 512 device columns per panel
SLOT = 512                  # PSUM bank-aligned token slot

# panel indices
G_XN, G_XT, G_XR, G_XTN = range(4)

_CACHE = {}


def _legalize_multi_waits(nc, max_waits=1):
    n_new = 0
    for f in nc.m.functions:
        for bb in f.blocks:
            new_list = []
            for inst in bb.instructions:
                si = inst.sync_info
                ow = list(si.on_wait) if si and si.on_wait else []
                if len(ow) > max_waits:
                    keep = ow[:max_waits]
                    for w in ow[max_waits:]:
                        nop = mybir.InstNoOp(name=f"I-mwsplit-{n_new}", ins=[], outs=[])
                        nop.engine = inst.engine
                        nop.sync_info = mybir.SyncInfo(on_wait=[w], on_update=[])
                        new_list.append(nop)
                        n_new += 1
                    inst.sync_info = mybir.SyncInfo(
                        on_wait=keep,
                        on_update=list(si.on_update) if si.on_update else [],
                    )
                new_list.append(inst)
            bb.instructions = new_list
    return n_new


# ---------------------------------------------------------------------------
# Pair tiling. Block = ('full', I, J) covering pairs (8I+a, 8J+c), a,c in 0..7
# or ('xrun', r, L) covering pairs (8t+r, 8t+r+1+v), t in 0..7, v in 0..L-1.
# Token order inside a block: (a,c,b) / (t,v,b); ntok = 64*8 / 8*L*8.
# ---------------------------------------------------------------------------

def _block_pairs(blk):
    kind = blk[0]
    if kind == 'full':
        _, I, J = blk
        return [(8 * I + a, 8 * J + c) for a in range(8) for c in range(8)]
    _, r, L = blk
    return [(8 * t + r, 8 * t + r + 1 + v) for t in range(8) for v in range(L)]


def _block_ntok(blk):
    return len(_block_pairs(blk)) * BC


def _triangle_tiles():
    tiles = []
    for I in range(8):
        for J in range(I + 1, 8):
            tiles.append([('full', I, J)])
    tiles.append([('xrun', 0, 7), ('xrun', 6, 1)])
    tiles.append([('xrun', 1, 6), ('xrun', 5, 2)])
    tiles.append([('xrun', 2, 5), ('xrun', 4, 3)])
    tiles.append([('xrun', 3, 4)])
    return tiles


def _tiling(idx_i, idx_j):
    """Returns (tiles, pair_order) where pair_order[k] = original pair id of
    the k-th device pair."""
    pdict = {(int(i), int(j)): p for p, (i, j) in enumerate(zip(idx_i, idx_j))}
    tiles = _triangle_tiles()
    try:
        order = [pdict[pr] for tl in tiles for blk in tl for pr in _block_pairs(blk)]
        assert len(order) == P_TOT == len(pdict)
    except (KeyError, AssertionError):
        raise NotImplementedError(
            "kernel requires the canonical all-(i<j) pair set")
    return tiles, np.asarray(order, np.int64)


def _build_program(tiles, zero_bias, legalize=True):
    nc = bass.Bass()
    pan_in = nc.declare_dram_parameter("pan", [D, NPANEL * NCOL_DEV], bf16, isOutput=False)
    oh_in = nc.declare_dram_parameter("oh", [5, NTOK], bf16, isOutput=False)
    w1_in = nc.declare_dram_parameter("w1", [D, 8 * 64], bf16, isOutput=False)
    mf_in = nc.declare_dram_parameter("mf", [D, 4 * 256], bf16, isOutput=False)
    pt_in = nc.declare_dram_parameter("pt", [5, 256], bf16, isOutput=False)
    w2_in = nc.declare_dram_parameter("w2", [D, 256], bf16, isOutput=False)
    bc_in = nc.declare_dram_parameter("bc", [D, 4], f32, isOutput=False)
    out = nc.declare_dram_parameter("out", [D, NTOK], bf16, isOutput=True)

    with tile.TileContext(nc) as tc:
        ctx = ExitStack()
        const = ctx.enter_context(tc.tile_pool(name="const", bufs=1))
        featp = ctx.enter_context(tc.tile_pool(name="featp", bufs=2))
        chunkp = ctx.enter_context(tc.tile_pool(name="chunkp", bufs=2))
        psz = ctx.enter_context(tc.tile_pool(name="psz", bufs=1, space="PSUM"))
        psu = ctx.enter_context(tc.tile_pool(name="psu", bufs=1, space="PSUM"))
        psop = ctx.enter_context(tc.tile_pool(name="psop", bufs=2, space="PSUM"))

        pan = const.tile([D, NPANEL * NCOL_DEV], bf16)
        nc.sync.dma_start(pan[:], pan_in[:])
        oh = const.tile([5, NTOK], bf16)
        nc.sync.dma_start(oh[:], oh_in[:])
        w1 = const.tile([D, 8 * 64], bf16)
        nc.sync.dma_start(w1[:], w1_in[:])
        mf = const.tile([D, 4 * 256], bf16)
        nc.sync.dma_start(mf[:], mf_in[:])
        pt = const.tile([5, 256], bf16)
        nc.sync.dma_start(pt[:], pt_in[:])
        w2 = const.tile([D, 256], bf16)
        nc.sync.dma_start(w2[:], w2_in[:])
        bc = const.tile([D, 4], f32)
        nc.sync.dma_start(bc[:], bc_in[:])

        # panel view [p, g, t, w, b]: element col = g*512 + (8t+w)*8 + b
        pv = pan[:].rearrange("p (g t w b) -> p g t w b",
                              g=NPANEL, t=8, w=8, b=BC)

        def sides(blk, gA, gB):
            """A-side (broadcast over v) and B-side APs [p, nu, nv, BC]."""
            kind = blk[0]
            if kind == 'full':
                _, I, J = blk
                nu, nv = 8, 8
                a = pv[:, gA, I, :, :]                   # [p,8,b]
                A = a[:, :, None, :].broadcast_to((D, nu, nv, BC))
                bb = pv[:, gB, J, :, :]                  # [p,8,b]
                Bx = bb[:, None, :, :].broadcast_to((D, nu, nv, BC))
            else:
                _, r, L = blk
                nu, nv = 8, L
                a = pv[:, gA, :, r, :]                   # [p,8,b]
                A = a[:, :, None, :].broadcast_to((D, nu, nv, BC))
                Bx = pv[:, gB, :, r + 1:r + 1 + L, :]    # [p,8,L,b]
            return A, Bx

        W = [w1[:, 64 * k:64 * (k + 1)] for k in range(6)]
        W3 = w1[:, 384:512]   # [W_sub | -W_sub]

        for ti, tl in enumerate(tiles):
            NT = sum(_block_ntok(b) for b in tl)
            t0 = sum(_block_ntok(b) for tt in tiles[:ti] for b in tt)

            fMC = featp.tile([D, 2 * NT], bf16, tag="fMC", name="fMC")
            fDD = featp.tile([D, 2 * NT], bf16, tag="fDD", name="fDD")
            fDA = featp.tile([D, 2 * NT], bf16, tag="fDA", name="fDA")
            fABS = featp.tile([D, NT], bf16, tag="fABS", name="fABS")

            po = 0
            for blk in tl:
                ntok = _block_ntok(blk)
                nu = 8
                nv = ntok // (8 * BC)

                def oview(t, slot):
                    o = t[:].rearrange("p (g n) -> p g n", g=2)[:, slot, po:po + ntok]
                    return o.rearrange("p (u v b) -> p u v b", u=nu, b=BC)

                A, Bx = sides(blk, G_XN, G_XN)          # cos
                nc.gpsimd.tensor_mul(oview(fMC, 0), A, Bx)
                A, Bx = sides(blk, G_XT, G_XT)          # mult
                nc.gpsimd.tensor_mul(oview(fMC, 1), A, Bx)
                A, Bx = sides(blk, G_XT, G_XR)          # dab
                nc.vector.tensor_mul(oview(fDD, 0), A, Bx)
                A, Bx = sides(blk, G_XR, G_XT)          # dba
                nc.vector.tensor_mul(oview(fDD, 1), A, Bx)
                A, Bx = sides(blk, G_XT, G_XTN)         # diff = A - B
                nc.vector.tensor_add(oview(fDA, 0), A, Bx)
                A, Bx = sides(blk, G_XT, G_XT)          # add
                nc.vector.tensor_add(oview(fDA, 1), A, Bx)
                po += ntok

            # |diff| on scalar
            nc.scalar.activation(fABS[:], fDA[:, 0:NT], AF.Abs)

            # ---- stage 1 (PSUM slots are bank-aligned at stride SLOT) ----
            zAB = psz.tile([D, 2 * SLOT], f32, tag="zAB", name="zAB")
            zCD = psz.tile([D, 2 * SLOT], f32, tag="zCD", name="zCD")
            # c0 = (cos, mult)
            nc.tensor.matmul(zAB[0:64, 0:NT], W[0], fMC[:, 0:NT],
                             start=True, stop=True, tile_position=(0, 0))
            nc.tensor.matmul(zAB[64:128, 0:NT], W[1], fMC[:, NT:2 * NT],
                             start=True, stop=True, tile_position=(0, 64))
            # c1 = (dab, dba)
            nc.tensor.matmul(zAB[0:64, SLOT:SLOT + NT], W[2], fDD[:, 0:NT],
                             start=True, stop=True, tile_position=(0, 0))
            nc.tensor.matmul(zAB[64:128, SLOT:SLOT + NT], W[3], fDD[:, NT:2 * NT],
                             start=True, stop=True, tile_position=(0, 64))
            # c2 = (absd, add)
            nc.tensor.matmul(zCD[0:64, 0:NT], W[4], fABS[:],
                             start=True, stop=True, tile_position=(0, 0))
            nc.tensor.matmul(zCD[64:128, 0:NT], W[5], fDA[:, NT:2 * NT],
                             start=True, stop=True, tile_position=(0, 64))
            # c3 = (sub, subba): one full-M matmul on the diff feature
            nc.tensor.matmul(zCD[:, SLOT:SLOT + NT], W3, fDA[:, 0:NT],
                             start=True, stop=True)

            # ---- chunk evictions ----
            ch = chunkp.tile([D, 4 * NT], bf16, tag="ch", name="ch")
            zABv = zAB[:].rearrange("p (s n) -> p s n", s=2)[:, :, 0:NT]
            zCDv = zCD[:].rearrange("p (s n) -> p s n", s=2)[:, :, 0:NT]
            chv = ch[:].rearrange("p (s n) -> p s n", s=4)
            if zero_bias:
                nc.scalar.activation(chv[:, 0:2, :], zABv, AF.Relu)
                nc.scalar.activation(chv[:, 2:4, :], zCDv, AF.Relu)
            else:
                for c in range(4):
                    z = (zAB, zCD)[c // 2]
                    zsl = z[:, (c % 2) * SLOT:(c % 2) * SLOT + NT]
                    nc.scalar.activation(ch[:, c * NT:(c + 1) * NT], zsl,
                                         AF.Relu, bias=bc[:, c:c + 1])

            # ---- stage 2 ----
            u = psu.tile([D, 2 * SLOT], f32, tag="u", name="u")
            for c in range(4):
                nc.tensor.matmul(u[:, 0:NT], mf[:, c * 256:c * 256 + 128],
                                 ch[:, c * NT:(c + 1) * NT],
                                 start=(c == 0), stop=False)
                nc.tensor.matmul(u[:, SLOT:SLOT + NT], mf[:, c * 256 + 128:(c + 1) * 256],
                                 ch[:, c * NT:(c + 1) * NT],
                                 start=(c == 0), stop=False)
            ohs = oh[:, t0:t0 + NT]
            nc.tensor.matmul(u[:, 0:NT], pt[:, 0:128], ohs, start=False, stop=True)
            nc.tensor.matmul(u[:, SLOT:SLOT + NT], pt[:, 128:256], ohs,
                             start=False, stop=True)

            # ---- relu(u): lo on scalar, hi on vector ----
            r = chunkp.tile([D, 2 * NT], bf16, tag="r", name="r")
            nc.scalar.activation(r[:, 0:NT], u[:, 0:NT], AF.Relu)
            nc.vector.tensor_scalar_max(r[:, NT:2 * NT], u[:, SLOT:SLOT + NT], 0.0)

            # ---- stage 3, evict to bf16, DMA out ----
            op = psop.tile([D, SLOT], f32, tag="op", name="op")
            nc.tensor.matmul(op[:, 0:NT], w2[:, 0:128], r[:, 0:NT],
                             start=True, stop=False)
            nc.tensor.matmul(op[:, 0:NT], w2[:, 128:256], r[:, NT:2 * NT],
                             start=False, stop=True)
            osb = chunkp.tile([D, NT], bf16, tag="osb", name="osb")
            nc.vector.tensor_copy(osb[:], op[:, 0:NT])
            nc.sync.dma_start(out[:, t0:t0 + NT], osb[:])
        ctx.close()

    if legalize:
        _legalize_multi_waits(nc)
    return nc


def _prep_host(x, presence, idx_i, idx_j,
               ops_W1, ops_b1, ops_W2, ops_b2,
               pres_W1, pres_b1, pres_W2, pres_b2,
               fus_W1, fus_b1, fus_W2, fus_b2):
    x = np.asarray(x, np.float32)
    ops_W1 = np.asarray(ops_W1, np.float32)
    ops_b1 = np.asarray(ops_b1, np.float32)
    ops_W2 = np.asarray(ops_W2, np.float32)
    ops_b2 = np.asarray(ops_b2, np.float32)
    fus_W1 = np.asarray(fus_W1, np.float32)
    fus_b1 = np.asarray(fus_b1, np.float32)
    fus_W2 = np.asarray(fus_W2, np.float32)
    pres_W1 = np.asarray(pres_W1, np.float32)
    pres_b1 = np.asarray(pres_b1, np.float32)
    pres_W2 = np.asarray(pres_W2, np.float32)
    pres_b2 = np.asarray(pres_b2, np.float32)

    idx_i = np.asarray(idx_i)
    idx_j = np.asarray(idx_j)
    _, pair_order = _tiling(idx_i, idx_j)

    # per-core panels, cols (c-major, b-inner); panel order xn,xT,xr,xT,-xT,xT
    norms = np.linalg.norm(x, axis=2)
    xn_full = x / (norms[:, :, None] + EPS)
    xr_full = 1.0 / (x + EPS)
    pans = []
    for k in range(NCORES):
        bs = slice(k * BC, (k + 1) * BC)

        def panel(a):
            return np.transpose(a[bs], (2, 1, 0)).reshape(D, NCOL_DEV)
        pT = panel(x)
        p4 = np.concatenate(
            [panel(xn_full), pT, panel(xr_full), -pT], axis=1)
        pans.append(np.ascontiguousarray(p4).astype(BF))

    # one-hot rows [ones, t0..t3] per core in DEVICE pair order
    pa = np.asarray(presence)[:, idx_i].astype(np.int64)   # [B, P]
    pb = np.asarray(presence)[:, idx_j].astype(np.int64)
    tsel = (2 * pa + pb)[:, pair_order]                    # device pair order
    ohs = []
    for k in range(NCORES):
        bs = slice(k * BC, (k + 1) * BC)
        tloc = np.transpose(tsel[bs], (1, 0)).reshape(NTOK)  # p-major, b-inner
        o = np.zeros((5, NTOK), np.float32)
        o[0] = 1.0
        for tt in range(4):
            o[1 + tt] = (tloc == tt)
        ohs.append(o.astype(BF))

    # slot tables: chunk slots (A=rows 0:64, B=rows 64:128)
    # c0=(cos,mult) c1=(dab,dba) c2=(absd,add) c3=(sub,subba)
    slot_blk = [2, 0, 6, 7, 3, 1, 4, 5]     # reference feature index per slot
    slot_w1 = [ops_W1[2], ops_W1[0], ops_W1[5], ops_W1[5],
               ops_W1[3], ops_W1[1], ops_W1[4], -ops_W1[4]]
    slot_b1 = [ops_b1[2], ops_b1[0], ops_b1[5], ops_b1[5],
               ops_b1[3], ops_b1[1], ops_b1[4], ops_b1[4]]
    slot_w2i = [MLP_IDX[b] for b in slot_blk]

    Wcat = np.concatenate(slot_w1, axis=1)                 # [128, 512]

    Mf_dev = np.zeros((128, 4 * 256), np.float32)
    c0row = fus_b1.copy()
    for s in range(8):
        blk = fus_W1[slot_blk[s] * D:(slot_blk[s] + 1) * D]  # [128, 256]
        m = ops_W2[slot_w2i[s]] @ blk                        # [64, 256]
        c = s // 2
        half = (s % 2) * 64
        Mf_dev[half:half + 64, c * 256:(c + 1) * 256] = m
        c0row += ops_b2[slot_w2i[s]] @ blk

    fus_blk_p = fus_W1[8 * D:]
    Mp = pres_W2 @ fus_blk_p
    ptabc = np.zeros((5, 256), np.float32)
    ptabc[0] = c0row + pres_b2 @ fus_blk_p
    for tt in range(4):
        hp = np.maximum(pres_W1[tt] + pres_b1, 0.0)
        ptabc[1 + tt] = hp @ Mp

    bcat = np.stack([np.concatenate([slot_b1[2 * c], slot_b1[2 * c + 1]])
                     for c in range(4)], axis=1).astype(np.float32)  # [128, 4]
    zero_bias = not (np.any(bcat) or False)

    w2_dev = np.concatenate([fus_W2[0:128], fus_W2[128:256]], axis=1)  # [128,256]

    shared = dict(
        w1=np.ascontiguousarray(Wcat).astype(BF),
        mf=np.ascontiguousarray(Mf_dev).astype(BF),
        pt=np.ascontiguousarray(ptabc).astype(BF),
        w2=np.ascontiguousarray(w2_dev).astype(BF),
        bc=np.ascontiguousarray(bcat),
    )
    in_maps = []
    for k in range(NCORES):
        m = dict(shared)
        m["pan"] = pans[k]
        m["oh"] = ohs[k]
        in_maps.append(m)
    return in_maps


def kernel(x, presence, idx_i, idx_j,
           ops_W1, ops_b1, ops_W2, ops_b2,
           pres_W1, pres_b1, pres_W2, pres_b2,
           fus_W1, fus_b1, fus_W2, fus_b2):
    idx_i = np.asarray(idx_i)
    idx_j = np.asarray(idx_j)

    in_maps = _prep_host(x, presence, idx_i, idx_j,
                         ops_W1, ops_b1, ops_W2, ops_b2,
                         pres_W1, pres_b1, pres_W2, pres_b2,
                         fus_W1, fus_b1, fus_W2, fus_b2)
    zero_bias = not np.any(np.asarray(ops_b1))
    key = (idx_i.tobytes(), idx_j.tobytes(), zero_bias)
    if key not in _CACHE:
        _CACHE.clear()
        tiles, _ = _tiling(idx_i, idx_j)
        _CACHE[key] = _build_program(tiles, zero_bias)
    nc = _CACHE[key]

    res = run_bass_kernel_spmd(nc, in_maps, core_ids=list(range(NCORES)))
    kernel._last_results = res

    _, pair_order = _tiling(idx_i, idx_j)
    fus_b2 = np.asarray(fus_b2, np.float32)

    # reassemble: device token t (in core k) = (device-pair dp, b_local),
    # original pair p = pair_order[dp]; out[k*BC+b, p, d] = core_out[d, t]
    out = np.empty((B, P_TOT, D), np.float32)
    inv = np.empty(P_TOT, np.int64)
    inv[pair_order] = np.arange(P_TOT)
    for k in range(NCORES):
        co = res.results[k]["out"]                  # [D, NTOK]
        co = co.reshape(D, P_TOT, BC)               # [d, device-pair, b_local]
        out[k * BC:(k + 1) * BC] = np.transpose(co, (2, 1, 0))[:, inv, :]
    out += fus_b2[None, None, :]
    return out.astype(np.asarray(x).dtype)


# revision 10
# speedup vs baseline: 1.1896x; 1.1896x over previous
"""DynamicRelationshipExtractor Trainium2 kernel (8 NeuronCores, batch-sharded).

Sharding: core k handles batches [8k, 8k+8) x all 2016 pairs (16128 tokens).
All cores run ONE SPMD program; per-core data enters via inputs. Pair indices
are baked into access patterns at trace time.

v2 restructure (vs the 180.8us baseline):
- Pairs re-ordered into 8x8 rectangular blocks (28 full blocks above the
  block-diagonal) plus cross-runs that batch the 8 diagonal triangles with a
  stride-64 AP dim. 32 tiles, 1-2 regular blocks each; feature elementwise
  work becomes a few large multi-dim-AP instructions per tile.
- Feature pairs sharing one ALU op are computed in single merged instructions
  over a panel-pair AP dim: (cos,mult) on GpSimd, (dab,dba) and (diff,add) on
  Vector. |diff| on Scalar. Panels [xn,xT,xr,xT,-xT,xT] are host-built so all
  pairs are forward-contiguous panel slices.
- sub/subba stage-1 is one full-M=128 matmul with lhsT [W_sub|-W_sub] reading
  the diff feature (full-M matmuls hide LDWEIGHTS; M=64 col-pairs don't).
- Chunk relu evictions merged into 2 activations of [128, 2*NT] per tile
  (zero-bias fast path; 4 split activations with bias APs otherwise).
- relu(u) eviction is one merged vector tensor_scalar over [128, 2*NT].
- Stage-3 accumulator goes PSUM -> DRAM by DMA directly (fus_b2 added on
  host), removing the final eviction pass.
"""
import sys
import numpy as np
import ml_dtypes
from contextlib import ExitStack

if '/opt/trn_rl_repo' not in sys.path:
    sys.path.insert(0, '/opt/trn_rl_repo')

import concourse.bass as bass
import concourse.tile as tile
from concourse import mybir
from concourse.bass_utils import run_bass_kernel_spmd

bf16 = mybir.dt.bfloat16
f32 = mybir.dt.float32
AF = mybir.ActivationFunctionType
ALU = mybir.AluOpType
BF = ml_dtypes.bfloat16

D = 128
NCOLS = 64
B = 64
P_TOT = 2016
EPS = 1e-6
MLP_IDX = [0, 1, 2, 3, 4, 4, 5, 5]
NCORES = 8
BC = B // NCORES            # batches per core = 8
NTOK = P_TOT * BC           # tokens per core = 16128
NPANEL = 4                  # xn, xT, xr, -xT
NCOL_DEV = NCOLS * BC       # 512 device columns per panel
SLOT = 512                  # PSUM bank-aligned token slot

# panel indices
G_XN, G_XT, G_XR, G_XTN = range(4)

_CACHE = {}


def _legalize_multi_waits(nc, max_waits=1):
    n_new = 0
    for f in nc.m.functions:
        for bb in f.blocks:
            new_list = []
            for inst in bb.instructions:
                si = inst.sync_info
                ow = list(si.on_wait) if si and si.on_wait else []
                if len(ow) > max_waits:
                    keep = ow[:max_waits]
                    for w in ow[max_waits:]:
                        nop = mybir.InstNoOp(name=f"I-mwsplit-{n_new}", ins=[], outs=[])
                        nop.engine = inst.engine
                        nop.sync_info = mybir.SyncInfo(on_wait=[w], on_update=[])
                        new_list.append(nop)
                        n_new += 1
                    inst.sync_info = mybir.SyncInfo(
                        on_wait=keep,
                        on_update=list(si.on_update) if si.on_update else [],
                    )
                new_list.append(inst)
            bb.instructions = new_list
    return n_new


# ---------------------------------------------------------------------------
# Pair tiling. Block = ('full', I, J) covering pairs (8I+a, 8J+c), a,c in 0..7
# or ('xrun', r, L) covering pairs (8t+r, 8t+r+1+v), t in 0..7, v in 0..L-1.
# Token order inside a block: (a,c,b) / (t,v,b); ntok = 64*8 / 8*L*8.
# ---------------------------------------------------------------------------

def _block_pairs(blk):
    kind = blk[0]
    if kind == 'full':
        _, I, J = blk
        return [(8 * I + a, 8 * J + c) for a in range(8) for c in range(8)]
    _, r, L = blk
    return [(8 * t + r, 8 * t + r + 1 + v) for t in range(8) for v in range(L)]


def _block_ntok(blk):
    return len(_block_pairs(blk)) * BC


def _triangle_tiles():
    tiles = []
    for I in range(8):
        for J in range(I + 1, 8):
            tiles.append([('full', I, J)])
    tiles.append([('xrun', 0, 7), ('xrun', 6, 1)])
    tiles.append([('xrun', 1, 6), ('xrun', 5, 2)])
    tiles.append([('xrun', 2, 5), ('xrun', 4, 3)])
    tiles.append([('xrun', 3, 4)])
    return tiles


def _tiling(idx_i, idx_j):
    """Returns (tiles, pair_order) where pair_order[k] = original pair id of
    the k-th device pair."""
    pdict = {(int(i), int(j)): p for p, (i, j) in enumerate(zip(idx_i, idx_j))}
    tiles = _triangle_tiles()
    try:
        order = [pdict[pr] for tl in tiles for blk in tl for pr in _block_pairs(blk)]
        assert len(order) == P_TOT == len(pdict)
    except (KeyError, AssertionError):
        raise NotImplementedError(
            "kernel requires the canonical all-(i<j) pair set")
    return tiles, np.asarray(order, np.int64)


def _build_program(tiles, zero_bias, legalize=True):
    nc = bass.Bass()
    pan_in = nc.declare_dram_parameter("pan", [D, NPANEL * NCOL_DEV], bf16, isOutput=False)
    oh_in = nc.declare_dram_parameter("oh", [5, NTOK], bf16, isOutput=False)
    w1_in = nc.declare_dram_parameter("w1", [D, 8 * 64], bf16, isOutput=False)
    mf_in = nc.declare_dram_parameter("mf", [D, 4 * 256], bf16, isOutput=False)
    pt_in = nc.declare_dram_parameter("pt", [5, 256], bf16, isOutput=False)
    w2_in = nc.declare_dram_parameter("w2", [D, 256], bf16, isOutput=False)
    bc_in = nc.declare_dram_parameter("bc", [D, 4], f32, isOutput=False)
    out = nc.declare_dram_parameter("out", [D, NTOK], bf16, isOutput=True)

    with tile.TileContext(nc) as tc:
        ctx = ExitStack()
        const = ctx.enter_context(tc.tile_pool(name="const", bufs=1))
        featp = ctx.enter_context(tc.tile_pool(name="featp", bufs=2))
        chunkp = ctx.enter_context(tc.tile_pool(name="chunkp", bufs=2))
        psz = ctx.enter_context(tc.tile_pool(name="psz", bufs=1, space="PSUM"))
        psu = ctx.enter_context(tc.tile_pool(name="psu", bufs=1, space="PSUM"))
        psop = ctx.enter_context(tc.tile_pool(name="psop", bufs=2, space="PSUM"))

        pan = const.tile([D, NPANEL * NCOL_DEV], bf16)
        nc.sync.dma_start(pan[:], pan_in[:])
        oh = const.tile([5, NTOK], bf16)
        nc.sync.dma_start(oh[:], oh_in[:])
        w1 = const.tile([D, 8 * 64], bf16)
        nc.sync.dma_start(w1[:], w1_in[:])
        mf = const.tile([D, 4 * 256], bf16)
        nc.sync.dma_start(mf[:], mf_in[:])
        pt = const.tile([5, 256], bf16)
        nc.sync.dma_start(pt[:], pt_in[:])
        w2 = const.tile([D, 256], bf16)
        nc.sync.dma_start(w2[:], w2_in[:])
        bc = const.tile([D, 4], f32)
        nc.sync.dma_start(bc[:], bc_in[:])

        # panel view [p, g, t, w, b]: element col = g*512 + (8t+w)*8 + b
        pv = pan[:].rearrange("p (g t w b) -> p g t w b",
                              g=NPANEL, t=8, w=8, b=BC)

        def sides(blk, gA, gB):
            """A-side (broadcast over v) and B-side APs [p, nu, nv, BC]."""
            kind = blk[0]
            if kind == 'full':
                _, I, J = blk
                nu, nv = 8, 8
                a = pv[:, gA, I, :, :]                   # [p,8,b]
                A = a[:, :, None, :].broadcast_to((D, nu, nv, BC))
                bb = pv[:, gB, J, :, :]                  # [p,8,b]
                Bx = bb[:, None, :, :].broadcast_to((D, nu, nv, BC))
            else:
                _, r, L = blk
                nu, nv = 8, L
                a = pv[:, gA, :, r, :]                   # [p,8,b]
                A = a[:, :, None, :].broadcast_to((D, nu, nv, BC))
                Bx = pv[:, gB, :, r + 1:r + 1 + L, :]    # [p,8,L,b]
            return A, Bx

        W = [w1[:, 64 * k:64 * (k + 1)] for k in range(6)]
        W3 = w1[:, 384:512]   # [W_sub | -W_sub]

        for ti, tl in enumerate(tiles):
            NT = sum(_block_ntok(b) for b in tl)
            t0 = sum(_block_ntok(b) for tt in tiles[:ti] for b in tt)

            fMC = featp.tile([D, 2 * NT], bf16, tag="fMC", name="fMC")
            fDD = featp.tile([D, 2 * NT], bf16, tag="fDD", name="fDD")
            fDA = featp.tile([D, 2 * NT], bf16, tag="fDA", name="fDA")
            fABS = featp.tile([D, NT], bf16, tag="fABS", name="fABS")

            po = 0
            for blk in tl:
                ntok = _block_ntok(blk)
                nu = 8
                nv = ntok // (8 * BC)

                def oview(t, slot):
                    o = t[:].rearrange("p (g n) -> p g n", g=2)[:, slot, po:po + ntok]
                    return o.rearrange("p (u v b) -> p u v b", u=nu, b=BC)

                A, Bx = sides(blk, G_XT, G_XTN)         # diff = A - B
                nc.vector.tensor_add(oview(fDA, 0), A, Bx)
                A, Bx = sides(blk, G_XT, G_XT)          # add
                nc.vector.tensor_add(oview(fDA, 1), A, Bx)
                A, Bx = sides(blk, G_XT, G_XR)          # dab
                nc.vector.tensor_mul(oview(fDD, 0), A, Bx)
                A, Bx = sides(blk, G_XR, G_XT)          # dba
                nc.vector.tensor_mul(oview(fDD, 1), A, Bx)
                A, Bx = sides(blk, G_XN, G_XN)          # cos
                nc.vector.tensor_mul(oview(fMC, 0), A, Bx)
                A, Bx = sides(blk, G_XT, G_XT)          # mult
                nc.vector.tensor_mul(oview(fMC, 1), A, Bx)
                po += ntok

            # |diff| on scalar
            nc.scalar.activation(fABS[:], fDA[:, 0:NT], AF.Abs)

            # ---- stage 1: per-chunk PSUM tiles, readiness order c1,c3,c0,c2
            zc = {c: psz.tile([D, SLOT], f32, tag=f"zc{c}", name=f"zc{c}")
                  for c in (1, 3, 0, 2)}
            # c1 = (dab, dba)
            nc.tensor.matmul(zc[1][0:64, 0:NT], W[2], fDD[:, 0:NT],
                             start=True, stop=True, tile_position=(0, 0))
            nc.tensor.matmul(zc[1][64:128, 0:NT], W[3], fDD[:, NT:2 * NT],
                             start=True, stop=True, tile_position=(0, 64))
            # c3 = (sub, subba): one full-M matmul on the diff feature
            nc.tensor.matmul(zc[3][:, 0:NT], W3, fDA[:, 0:NT],
                             start=True, stop=True)
            # c0 = (cos, mult)
            nc.tensor.matmul(zc[0][0:64, 0:NT], W[0], fMC[:, 0:NT],
                             start=True, stop=True, tile_position=(0, 0))
            nc.tensor.matmul(zc[0][64:128, 0:NT], W[1], fMC[:, NT:2 * NT],
                             start=True, stop=True, tile_position=(0, 64))
            # c2 = (absd, add)
            nc.tensor.matmul(zc[2][0:64, 0:NT], W[4], fABS[:],
                             start=True, stop=True, tile_position=(0, 0))
            nc.tensor.matmul(zc[2][64:128, 0:NT], W[5], fDA[:, NT:2 * NT],
                             start=True, stop=True, tile_position=(0, 64))

            # ---- chunk evictions (scalar), readiness order ----
            ch = chunkp.tile([D, 4 * NT], bf16, tag="ch", name="ch")
            for c in (1, 3, 0, 2):
                kw = {} if zero_bias else {"bias": bc[:, c:c + 1]}
                nc.scalar.activation(ch[:, c * NT:(c + 1) * NT],
                                     zc[c][:, 0:NT], AF.Relu, **kw)

            # ---- stage 2 ----
            u_lo = psu.tile([D, SLOT], f32, tag="u_lo", name="u_lo")
            u_hi = psu.tile([D, SLOT], f32, tag="u_hi", name="u_hi")
            for ci, c in enumerate((1, 3, 0, 2)):
                nc.tensor.matmul(u_lo[:, 0:NT], mf[:, c * 256:c * 256 + 128],
                                 ch[:, c * NT:(c + 1) * NT],
                                 start=(ci == 0), stop=False)
                nc.tensor.matmul(u_hi[:, 0:NT], mf[:, c * 256 + 128:(c + 1) * 256],
                                 ch[:, c * NT:(c + 1) * NT],
                                 start=(ci == 0), stop=False)
            ohs = oh[:, t0:t0 + NT]
            nc.tensor.matmul(u_lo[:, 0:NT], pt[:, 0:128], ohs, start=False, stop=True)
            nc.tensor.matmul(u_hi[:, 0:NT], pt[:, 128:256], ohs,
                             start=False, stop=True)

            # ---- relu(u): lo on scalar, hi on vector ----
            r = chunkp.tile([D, 2 * NT], bf16, tag="r", name="r")
            nc.scalar.activation(r[:, 0:NT], u_lo[:, 0:NT], AF.Relu)
            nc.vector.tensor_scalar_max(r[:, NT:2 * NT], u_hi[:, 0:NT], 0.0)

            # ---- stage 3, evict to bf16, DMA out ----
            op = psop.tile([D, SLOT], f32, tag="op", name="op")
            nc.tensor.matmul(op[:, 0:NT], w2[:, 0:128], r[:, 0:NT],
                             start=True, stop=False)
            nc.tensor.matmul(op[:, 0:NT], w2[:, 128:256], r[:, NT:2 * NT],
                             start=False, stop=True)
            osb = chunkp.tile([D, NT], bf16, tag="osb", name="osb")
            nc.vector.tensor_copy(osb[:], op[:, 0:NT])
            nc.sync.dma_start(out[:, t0:t0 + NT], osb[:])
        ctx.close()

    if legalize:
        _legalize_multi_waits(nc)
    return nc


def _prep_host(x, presence, idx_i, idx_j,
               ops_W1, ops_b1, ops_W2, ops_b2,
               pres_W1, pres_b1, pres_W2, pres_b2,
               fus_W1, fus_b1, fus_W2, fus_b2):
    x = np.asarray(x, np.float32)
    ops_W1 = np.asarray(ops_W1, np.float32)
    ops_b1 = np.asarray(ops_b1, np.float32)
    ops_W2 = np.asarray(ops_W2, np.float32)
    ops_b2 = np.asarray(ops_b2, np.float32)
    fus_W1 = np.asarray(fus_W1, np.float32)
    fus_b1 = np.asarray(fus_b1, np.float32)
    fus_W2 = np.asarray(fus_W2, np.float32)
    pres_W1 = np.asarray(pres_W1, np.float32)
    pres_b1 = np.asarray(pres_b1, np.float32)
    pres_W2 = np.asarray(pres_W2, np.float32)
    pres_b2 = np.asarray(pres_b2, np.float32)

    idx_i = np.asarray(idx_i)
    idx_j = np.asarray(idx_j)
    _, pair_order = _tiling(idx_i, idx_j)

    # per-core panels, cols (c-major, b-inner); panel order xn,xT,xr,xT,-xT,xT
    norms = np.linalg.norm(x, axis=2)
    xn_full = x / (norms[:, :, None] + EPS)
    xr_full = 1.0 / (x + EPS)
    pans = []
    for k in range(NCORES):
        bs = slice(k * BC, (k + 1) * BC)

        def panel(a):
            return np.transpose(a[bs], (2, 1, 0)).reshape(D, NCOL_DEV)
        pT = panel(x)
        p4 = np.concatenate(
            [panel(xn_full), pT, panel(xr_full), -pT], axis=1)
        pans.append(np.ascontiguousarray(p4).astype(BF))

    # one-hot rows [ones, t0..t3] per core in DEVICE pair order
    pa = np.asarray(presence)[:, idx_i].astype(np.int64)   # [B, P]
    pb = np.asarray(presence)[:, idx_j].astype(np.int64)
    tsel = (2 * pa + pb)[:, pair_order]                    # device pair order
    ohs = []
    for k in range(NCORES):
        bs = slice(k * BC, (k + 1) * BC)
        tloc = np.transpose(tsel[bs], (1, 0)).reshape(NTOK)  # p-major, b-inner
        o = np.zeros((5, NTOK), np.float32)
        o[0] = 1.0
        for tt in range(4):
            o[1 + tt] = (tloc == tt)
        ohs.append(o.astype(BF))

    # slot tables: chunk slots (A=rows 0:64, B=rows 64:128)
    # c0=(cos,mult) c1=(dab,dba) c2=(absd,add) c3=(sub,subba)
    slot_blk = [2, 0, 6, 7, 3, 1, 4, 5]     # reference feature index per slot
    slot_w1 = [ops_W1[2], ops_W1[0], ops_W1[5], ops_W1[5],
               ops_W1[3], ops_W1[1], ops_W1[4], -ops_W1[4]]
    slot_b1 = [ops_b1[2], ops_b1[0], ops_b1[5], ops_b1[5],
               ops_b1[3], ops_b1[1], ops_b1[4], ops_b1[4]]
    slot_w2i = [MLP_IDX[b] for b in slot_blk]

    Wcat = np.concatenate(slot_w1, axis=1)                 # [128, 512]

    Mf_dev = np.zeros((128, 4 * 256), np.float32)
    c0row = fus_b1.copy()
    for s in range(8):
        blk = fus_W1[slot_blk[s] * D:(slot_blk[s] + 1) * D]  # [128, 256]
        m = ops_W2[slot_w2i[s]] @ blk                        # [64, 256]
        c = s // 2
        half = (s % 2) * 64
        Mf_dev[half:half + 64, c * 256:(c + 1) * 256] = m
        c0row += ops_b2[slot_w2i[s]] @ blk

    fus_blk_p = fus_W1[8 * D:]
    Mp = pres_W2 @ fus_blk_p
    ptabc = np.zeros((5, 256), np.float32)
    ptabc[0] = c0row + pres_b2 @ fus_blk_p
    for tt in range(4):
        hp = np.maximum(pres_W1[tt] + pres_b1, 0.0)
        ptabc[1 + tt] = hp @ Mp

    bcat = np.stack([np.concatenate([slot_b1[2 * c], slot_b1[2 * c + 1]])
                     for c in range(4)], axis=1).astype(np.float32)  # [128, 4]
    zero_bias = not (np.any(bcat) or False)

    w2_dev = np.concatenate([fus_W2[0:128], fus_W2[128:256]], axis=1)  # [128,256]

    shared = dict(
        w1=np.ascontiguousarray(Wcat).astype(BF),
        mf=np.ascontiguousarray(Mf_dev).astype(BF),
        pt=np.ascontiguousarray(ptabc).astype(BF),
        w2=np.ascontiguousarray(w2_dev).astype(BF),
        bc=np.ascontiguousarray(bcat),
    )
    in_maps = []
    for k in range(NCORES):
        m = dict(shared)
        m["pan"] = pans[k]
        m["oh"] = ohs[k]
        in_maps.append(m)
    return in_maps


def kernel(x, presence, idx_i, idx_j,
           ops_W1, ops_b1, ops_W2, ops_b2,
           pres_W1, pres_b1, pres_W2, pres_b2,
           fus_W1, fus_b1, fus_W2, fus_b2):
    idx_i = np.asarray(idx_i)
    idx_j = np.asarray(idx_j)

    in_maps = _prep_host(x, presence, idx_i, idx_j,
                         ops_W1, ops_b1, ops_W2, ops_b2,
                         pres_W1, pres_b1, pres_W2, pres_b2,
                         fus_W1, fus_b1, fus_W2, fus_b2)
    zero_bias = not np.any(np.asarray(ops_b1))
    key = (idx_i.tobytes(), idx_j.tobytes(), zero_bias)
    if key not in _CACHE:
        _CACHE.clear()
        tiles, _ = _tiling(idx_i, idx_j)
        _CACHE[key] = _build_program(tiles, zero_bias)
    nc = _CACHE[key]

    res = run_bass_kernel_spmd(nc, in_maps, core_ids=list(range(NCORES)))
    kernel._last_results = res

    _, pair_order = _tiling(idx_i, idx_j)
    fus_b2 = np.asarray(fus_b2, np.float32)

    # reassemble: device token t (in core k) = (device-pair dp, b_local),
    # original pair p = pair_order[dp]; out[k*BC+b, p, d] = core_out[d, t]
    out = np.empty((B, P_TOT, D), np.float32)
    inv = np.empty(P_TOT, np.int64)
    inv[pair_order] = np.arange(P_TOT)
    for k in range(NCORES):
        co = res.results[k]["out"]                  # [D, NTOK]
        co = co.reshape(D, P_TOT, BC)               # [d, device-pair, b_local]
        out[k * BC:(k + 1) * BC] = np.transpose(co, (2, 1, 0))[:, inv, :]
    out += fus_b2[None, None, :]
    return out.astype(np.asarray(x).dtype)
